# revision 26
# baseline (speedup 1.0000x reference)
"""Trainium2 Bass kernel for nn_CLinear_6768868459230 (V-formulation).

Context-conditioned block-autoregressive linear layer (MAF-style):
  wdir = c @ Wd + bd                      [B, O, I]
  w    = exp(wdir)*mask_diag + wdir*mask_lower
  sqn  = sum(w^2, axis=I)
  y    = (w / sqrt(sqn) * exp(wamp)) @ xv + bias
  logdet = logsumexp over diag block of (wdir - 0.5 log sqn + wamp + xl)

Sharding: tensor-parallel over the O=512 output rows. Each of the 8 cores
owns 8 of the 64 channels, chosen as {k, 15-k, 16+k, 31-k, ...} so the
per-slot lower-width window count is identical on every core (shared
program).

Per row r the only heavy quantities are the two lower-block reductions
  B_r[b] = sum_i t[b,r,i]*xv[b,i]   and   A_r[b] = sum_i t[b,r,i]^2
with t = c @ W_r. Neither requires materializing t in batch-major layout:

  B_r = c . V_r           V_r = W_r @ xv   (PE, contract over i in
                          128-wide windows, accumulate in PSUM; [128k, 256b])
  A_r = ||L_r^T c||^2     L_r from a host-side Cholesky of the Gram
                          W_r W_r^T (compresses rows wider than 128), plus
                          host-folded cross/const terms for bd.

One DVE multiply (V*cT) and one ScalarE square (z^2) fill a combined
bf16 tile per row pair; a single 512-wide one-hot selector matmul per row
then reduces both quantities over the 128 partitions straight into the
PSUM accumulators (sqn|dot). All reductions run on the PE at 128-way
parallelism; the batch is one 256-wide free dim; PSUM start/stop follows
a one-start-per-bank discipline (2KB zero regions).

DMA rides all four DGE queues (sync/scalar/gpsimd/vector) ordered so the
PE never waits: a single combined warmup tensor gates the first matmuls,
weight streams are split to arrive just ahead of consumption.
"""

import numpy as np

NCH, FIN, FOUT, CDIM, B = 64, 8, 8, 128, 256
I = NCH * FIN
O = NCH * FOUT
NCORES = 8
NLOC = 64  # output rows per core
WIN_OF_J = [1, 1, 2, 2, 3, 3, 4, 4]  # 128-wide i-windows per slot
NBLK = 8 * sum(WIN_OF_J)  # 160 weight blocks per core

# warmup tensor column offsets (bf16, [128, 960])
# BDT: per-diag-block bdd bias (cols 0-3) and 2*bdd (cols 4-7)
# SCL: per-row scalars on partitions 0-63: ebc | ba | bb
WM_CT, WM_WDD, WM_WAMP, WM_BIAS, WM_BDT, WM_SCL, WM_END = (
    0, 256, 768, 832, 896, 904, 1032)
# dx2 tensor column offsets (bf16, [128, 2688])
DX_XVD, DX_EXL, DX_BDM, DX_SEL16, DX_CROSS, DX_EBC, DX_END = (
    0, 1024, 2048, 2304, 2560, 2624, 2688)


def _channels(k):
    return [k, 15 - k, 16 + k, 31 - k, 32 + k, 47 - k, 48 + k, 63 - k]


_NC = None


def _build_nc():
    import concourse.bacc as bacc
    import concourse.tile as tile
    from concourse import mybir
    from concourse.masks import make_identity

    f32 = mybir.dt.float32
    bf16 = mybir.dt.bfloat16
    AF = mybir.ActivationFunctionType

    nc = bacc.Bacc(None, target_bir_lowering=False)

    d_warm = nc.dram_tensor("warm", [128, WM_END], bf16, kind="ExternalInput")
    d_xvt = nc.dram_tensor("xvt", [128, 4 * B], bf16, kind="ExternalInput")
    d_wtk = nc.dram_tensor("wtk", [128, NBLK * 128], bf16, kind="ExternalInput")
    d_lch = nc.dram_tensor("lch", [128, NLOC * 128], bf16, kind="ExternalInput")
    f8 = mybir.dt.float8e4
    d_sel64 = nc.dram_tensor("sel64", [128, 64 * 64], bf16, kind="ExternalInput")
    d_sel128 = nc.dram_tensor("sel128", [128, 32 * 2 * 64], f8, kind="ExternalInput")
    d_dx2 = nc.dram_tensor("dx2", [128, DX_END], bf16, kind="ExternalInput")
    d_out = nc.dram_tensor("out", [B, NLOC, 2], f32, kind="ExternalOutput")

    blk_start = [8 * sum(WIN_OF_J[:j]) for j in range(8)]

    with tile.TileContext(nc) as tc:
        with (
            tc.tile_pool(name="consts", bufs=1) as consts,
            tc.tile_pool(name="scrP", bufs=8) as scrP,
            tc.tile_pool(name="scrD", bufs=2) as scrD,
            tc.tile_pool(name="asm", bufs=1) as asm,
            tc.tile_pool(name="pVp", bufs=2, space="PSUM") as pVp,
            tc.tile_pool(name="pZp", bufs=2, space="PSUM") as pZp,
            tc.tile_pool(name="paccp", bufs=1, space="PSUM") as paccp,
            tc.tile_pool(name="ptdp", bufs=1, space="PSUM") as ptdp,
            tc.tile_pool(name="pexp", bufs=1, space="PSUM") as pexp,
        ):
            # ---- DMAs across four DGE queues ----
            warm_sb = consts.tile([128, WM_END], bf16)
            nc.sync.dma_start(out=warm_sb, in_=d_warm[:, :])
            xvt_sb = consts.tile([128, 4, B], bf16)
            nc.scalar.dma_start(out=xvt_sb, in_=d_xvt[:, :])
            wtk_sb = consts.tile([128, NBLK, 128], bf16)
            for j in range(6):  # scalar queue: slots 0-5
                b0, b1 = blk_start[j], blk_start[j] + 8 * WIN_OF_J[j]
                nc.scalar.dma_start(
                    out=wtk_sb[:, b0:b1, :], in_=d_wtk[:, b0 * 128 : b1 * 128]
                )
            lch_sb = consts.tile([128, NLOC, 128], bf16)
            for g in range(4):  # gpsimd queue: lch first
                nc.gpsimd.dma_start(
                    out=lch_sb[:, 16 * g : 16 * (g + 1), :],
                    in_=d_lch[:, 16 * g * 128 : 16 * (g + 1) * 128],
                )
            for j in (6, 7):  # then slots 6-7
                b0, b1 = blk_start[j], blk_start[j] + 8 * WIN_OF_J[j]
                nc.gpsimd.dma_start(
                    out=wtk_sb[:, b0:b1, :], in_=d_wtk[:, b0 * 128 : b1 * 128]
                )
            sel64_sb = consts.tile([128, 64, 64], bf16)
            nc.sync.dma_start(out=sel64_sb[:, 0:16, :], in_=d_sel64[:, 0 : 16 * 64])
            dx2_sb = consts.tile([128, DX_END], bf16)
            nc.sync.dma_start(out=dx2_sb, in_=d_dx2[:, :])
            sel128_sb = consts.tile([128, 32, 2, 64], f8)
            nc.sync.dma_start(out=sel128_sb, in_=d_sel128[:, :])
            nc.sync.dma_start(
                out=sel64_sb[:, 16:64, :], in_=d_sel64[:, 16 * 64 : 64 * 64]
            )

            ct_sb = warm_sb[:, WM_CT : WM_CT + 256]
            wdd_sb = warm_sb[:, WM_WDD : WM_WDD + 512]
            wampq_sb = warm_sb[:, WM_WAMP : WM_WAMP + 64]
            biasq_sb = warm_sb[:, WM_BIAS : WM_BIAS + 64]
            bdt_sb = warm_sb[:, WM_BDT : WM_BDT + 8]
            ebc_row = warm_sb[0:1, WM_SCL : WM_SCL + 64]
            ebb_row = warm_sb[0:1, WM_SCL + 64 : WM_SCL + 128]

            ones1 = consts.tile([1, B], bf16)
            nc.vector.memset(ones1, 1.0)
            onz = consts.tile([1, 512], bf16)
            nc.vector.memset(onz, 0.0)
            nc.vector.memset(onz[:, 0:256], 1.0)
            id64 = consts.tile([64, 64], f32)
            make_identity(nc, id64)

            ct_b2 = ct_sb.unsqueeze(1).broadcast_to([CDIM, 2, B])

            # ---- acc regions: bank0 = (sqn | dot), bank1 = (LDS | wamp) ----
            acc = paccp.tile([64, 4, 256], f32, name="acc", tag="acc")

            # warmup: wamp -> acc[:,3] (opens bank1); bias -> pex -> SBUF
            nc.tensor.matmul(
                acc[:, 3, :], wampq_sb, ct_sb, start=True, stop=False,
                skip_group_check=True,
            )
            pex = pexp.tile([128, 256], f32, name="pex", tag="pex")
            nc.tensor.matmul(
                pex[0:64, :], biasq_sb, ct_sb, start=True, stop=False,
                skip_group_check=True,
            )
            nc.tensor.matmul(
                pex[0:64, :], ebb_row, ones1, start=False, stop=True,
                skip_group_check=True,
            )
            bias_sb = asm.tile([64, 256], f32, name="bias", tag="bias")
            nc.scalar.copy(bias_sb, pex[0:64, :])

            # open acc bank0 (sqn|dot) with one matmul covering the full
            # 2KB zero region; it also seeds sqn with the bd-const term.
            nc.tensor.matmul(
                acc[:, 0:2, :], ebc_row, onz,
                start=True, stop=False, skip_group_check=True,
            )

            # diag produce + exps (PE warmup while wtk streams in)
            E_g, E2_g = [], []
            for g in range(2):
                ptd = ptdp.tile([128, 2, 256], f32, name="ptd", tag="ptd")
                for u in range(2):
                    a = 2 * g + u
                    nc.tensor.matmul(
                        ptd[:, u, :],
                        wdd_sb[:, a * 128 : (a + 1) * 128],
                        ct_sb,
                        start=(u == 0),
                        stop=(u == 1),
                        skip_group_check=True,
                    )
                E = scrD.tile([128, 2, 256], bf16, name="E", tag="E")
                E2 = scrD.tile([128, 2, 256], bf16, name="E2", tag="E2")
                for u in range(2):
                    a = 2 * g + u
                    nc.scalar.activation(
                        out=E[:, u, :], in_=ptd[:, u, :], func=AF.Exp,
                        bias=bdt_sb[:, a : a + 1],
                    )
                    nc.scalar.activation(
                        out=E2[:, u, :], in_=ptd[:, u, :], func=AF.Exp,
                        scale=2.0, bias=bdt_sb[:, 4 + a : 5 + a],
                    )
                E_g.append(E)
                E2_g.append(E2)

            # diag products for dot/LDS (DVE) are emitted mid-loop (t==8,
            # once dx2 has landed) so they don't stall the comb TT stream;
            # their selector matmuls ride `late`.
            Pd_g, Pl_g = [], []

            def emit_diag_products():
                for g in range(2):
                    xvd_w = dx2_sb[:, DX_XVD + 512 * g : DX_XVD + 512 * (g + 1)]
                    exl_w = dx2_sb[:, DX_EXL + 512 * g : DX_EXL + 512 * (g + 1)]
                    Pd = scrD.tile([128, 2, 256], bf16, name="Pd", tag="Pd")
                    nc.vector.tensor_mul(
                        Pd, E_g[g], xvd_w.rearrange("p (u b) -> p u b", b=256)
                    )
                    Pl = scrD.tile([128, 2, 256], bf16, name="Pl", tag="Pl")
                    nc.vector.tensor_mul(
                        Pl, E_g[g], exl_w.rearrange("p (u b) -> p u b", b=256)
                    )
                    Pd_g.append(Pd)
                    Pl_g.append(Pl)

            crossq_sb = dx2_sb[:, DX_CROSS : DX_CROSS + 64]
            late = []
            late.append(lambda: nc.tensor.matmul(
                acc[:, 0, :], crossq_sb, ct_sb, start=False, stop=False,
                skip_group_check=True))
            for a_ in range(4):
                late.append(lambda a=a_: nc.tensor.matmul(
                    acc[:, 1, :],
                    dx2_sb[:, DX_BDM + 64 * a : DX_BDM + 64 * (a + 1)],
                    xvt_sb[:, a, :],
                    start=False, stop=False, skip_group_check=True))
            for g in range(2):
                for u in range(2):
                    a_ = 2 * g + u
                    late.append(lambda g=g, u=u, a=a_: nc.tensor.matmul(
                        acc[:, 0, :],
                        dx2_sb[:, DX_SEL16 + 64 * a : DX_SEL16 + 64 * (a + 1)],
                        E2_g[g][:, u, :],
                        start=False, stop=False, skip_group_check=True))
                    late.append(lambda g=g, u=u, a=a_: nc.tensor.matmul(
                        acc[:, 1, :],
                        dx2_sb[:, DX_SEL16 + 64 * a : DX_SEL16 + 64 * (a + 1)],
                        Pd_g[g][:, u, :],
                        start=False, stop=False, skip_group_check=True))
                    late.append(lambda g=g, u=u, a=a_: nc.tensor.matmul(
                        acc[:, 2, :],
                        dx2_sb[:, DX_SEL16 + 64 * a : DX_SEL16 + 64 * (a + 1)],
                        Pl_g[g][:, u, :],
                        start=False, stop=(a == 3), skip_group_check=True))

            # ---- merged pair loop ----
            blk = 0
            pend = []
            for t in range(32):
                pV = pVp.tile([128, 2, 256], f32, name="pV", tag="pV")
                for q in range(2):
                    r = 2 * t + q
                    nw = WIN_OF_J[r // 8]
                    for a in range(nw):
                        nc.tensor.matmul(
                            pV[:, q, :],
                            wtk_sb[:, blk, :],
                            xvt_sb[:, a, :],
                            start=(q == 0 and a == 0),
                            stop=(q == 1 and a == nw - 1),
                            skip_group_check=True,
                        )
                        blk += 1
                pz = pZp.tile([128, 2, 256], f32, name="pz", tag="pz")
                for q in range(2):
                    r = 2 * t + q
                    nc.tensor.matmul(
                        pz[:, q, :], lch_sb[:, r, :], ct_sb,
                        start=(q == 0), stop=(q == 1), skip_group_check=True,
                    )
                P = scrP.tile([128, 2, B], bf16, name="P", tag="P")
                nc.vector.tensor_mul(P, pV, ct_b2)
                z8 = scrP.tile([128, 2, B], f8, name="z8", tag="z8")
                nc.scalar.activation(out=z8, in_=pz, func=AF.Square, scale=4.0)
                if t == 8:
                    emit_diag_products()
                pend.append((t, P, z8))
                if len(pend) > 4:
                    tq, Pq, zq = pend.pop(0)
                    for q in range(2):
                        r = 2 * tq + q
                        nc.tensor.matmul(
                            acc[:, 1, :],
                            sel64_sb[:, r, :],
                            Pq[:, q, :],
                            start=False,
                            stop=False,
                            skip_group_check=True,
                        )
                    nc.tensor.matmul(
                        acc[:, 0, :],
                        sel128_sb[:, tq, :, :],
                        zq,
                        perf_mode=mybir.MatmulPerfMode.DoubleRow,
                        start=False,
                        stop=False,
                        skip_group_check=True,
                    )
                    if t >= 12 and late:
                        late.pop(0)()
                        if late:
                            late.pop(0)()
            for tq, Pq, zq in pend:
                for q in range(2):
                    r = 2 * tq + q
                    nc.tensor.matmul(
                        acc[:, 1, :],
                        sel64_sb[:, r, :],
                        Pq[:, q, :],
                        start=False,
                        stop=False,
                        skip_group_check=True,
                    )
                nc.tensor.matmul(
                    acc[:, 0, :],
                    sel128_sb[:, tq, :, :],
                    zq,
                    perf_mode=mybir.MatmulPerfMode.DoubleRow,
                    start=False,
                    stop=(tq == 31),
                    skip_group_check=True,
                )
            for fn in late:
                fn()
            late = []

            # ---- assembly (all [64, 256]); ld finishes first so its
            # transposes overlap the yb chain ----
            l1 = asm.tile([64, 256], f32, name="l1", tag="l1")
            nc.scalar.activation(out=l1, in_=acc[:, 0, :], func=AF.Ln)
            l2 = asm.tile([64, 256], f32, name="l2", tag="l2")
            nc.scalar.activation(out=l2, in_=acc[:, 2, :], func=AF.Ln)
            mh = asm.tile([64, 256], f32, name="mh", tag="mh")
            nc.vector.tensor_scalar_mul(mh, l1, -0.5)
            u_t = asm.tile([64, 256], f32, name="u", tag="u")
            nc.vector.tensor_add(u_t, acc[:, 3, :], mh)
            ld = asm.tile([64, 256], f32, name="ld", tag="ld")
            nc.vector.tensor_add(ld, u_t, l2)
            sc = asm.tile([64, 256], f32, name="sc", tag="sc")
            nc.scalar.activation(out=sc, in_=u_t, func=AF.Exp)
            pT = pexp.tile([128, 256], f32, name="pex", tag="pex")
            for i in range(2):
                nc.tensor.matmul(
                    pT[:, 128 + 64 * i : 192 + 64 * i],
                    ld[:, i * 128 : (i + 1) * 128],
                    id64,
                    is_transpose=True,
                    start=(i == 0),
                    stop=False,
                    skip_group_check=True,
                )
            yv = asm.tile([64, 256], f32, name="yv", tag="yv")
            nc.vector.tensor_mul(yv, acc[:, 1, :], sc)
            yb = asm.tile([64, 256], f32, name="yb", tag="yb")
            nc.vector.tensor_add(yb, yv, bias_sb)
            for i in range(2):
                nc.tensor.matmul(
                    pT[:, 64 * i : 64 * i + 64],
                    yb[:, i * 128 : (i + 1) * 128],
                    id64,
                    is_transpose=True,
                    start=False,
                    stop=(i == 1),
                    skip_group_check=True,
                )
            for h in range(2):
                ob = asm.tile([128, 64, 2], f32, name=f"ob{h}", tag=f"ob{h}")
                nc.vector.tensor_copy(out=ob[:, :, 0], in_=pT[:, 64 * h : 64 * h + 64])
                nc.vector.tensor_copy(
                    out=ob[:, :, 1], in_=pT[:, 128 + 64 * h : 192 + 64 * h]
                )
                nc.sync.dma_start(out=d_out[128 * h : 128 * (h + 1), :, :], in_=ob)

    nc.compile()
    return nc


def _host_prep(x, c, Wd, bd, Wa, ba, Wb, bb):
    """Build the 8 per-core input maps."""
    import ml_dtypes

    bf = ml_dtypes.bfloat16
    x = np.ascontiguousarray(x, dtype=np.float32)
    c = np.ascontiguousarray(c, dtype=np.float32)
    Wd5 = np.ascontiguousarray(Wd, dtype=np.float32).reshape(CDIM, NCH, FOUT, NCH, FIN)
    bd4 = np.ascontiguousarray(bd, dtype=np.float32).reshape(NCH, FOUT, NCH, FIN)
    Wa = np.asarray(Wa, dtype=np.float32)
    Wb = np.asarray(Wb, dtype=np.float32)
    ba = np.asarray(ba, dtype=np.float32)
    bb = np.asarray(bb, dtype=np.float32)

    cT = np.ascontiguousarray(c.T)  # [128, 256]
    xv = x[:, :, 0]
    xl = x[:, :, 1]
    xvT = np.ascontiguousarray(xv.T)  # [512, 256]
    exlT = np.exp(xl).T  # [512, 256]

    def fold4(a512):  # [512, N] -> [128, 4*N] window-major per partition
        N = a512.shape[1]
        return np.ascontiguousarray(
            a512.reshape(4, 128, N).transpose(1, 0, 2).reshape(128, 4 * N)
        )

    xvt = fold4(xvT).astype(bf)

    sel16 = np.zeros((128, 4, 64), dtype=np.float32)
    p = np.arange(128)
    for a in range(4):
        sel16[p, a, 16 * a + p // 8] = 1.0
    # sel64 carries exp(ba_r) per column (ba folded out of the device math)
    f8 = ml_dtypes.float8_e4m3fn
    sel128 = np.zeros((32, 2, 64), dtype=np.float32)
    for t_ in range(32):
        sel128[t_, 0, 2 * t_] = 0.0625
        sel128[t_, 1, 2 * t_ + 1] = 0.0625
    sel128 = np.ascontiguousarray(
        np.broadcast_to(sel128, (128, 32, 2, 64)).reshape(128, 4096)
    ).astype(f8)

    in_maps = []
    for k in range(NCORES):
        chs = _channels(k)
        blocks = []
        lch = np.zeros((128, NLOC, 128), dtype=np.float32)
        crossq = np.zeros((128, 64), dtype=np.float32)
        ebc = np.zeros((64,), dtype=np.float32)
        bdm = np.zeros((512, 64), dtype=np.float32)
        wdd = np.empty((128, 512), dtype=np.float32)
        bdd = np.zeros((512,), dtype=np.float32)
        xvd_cols = np.empty((512, B), dtype=np.float32)
        exl_cols = np.empty((512, B), dtype=np.float32)
        wampq = np.empty((128, 64), dtype=np.float32)
        biasq = np.empty((128, 64), dtype=np.float32)
        eba = np.zeros((64,), dtype=np.float32)
        ebb = np.zeros((64,), dtype=np.float32)

        eba_full = np.concatenate(
            [ba[ch * FOUT : (ch + 1) * FOUT] for ch in chs]
        )  # [64] per-row ba
        sexp = np.exp(eba_full).astype(np.float32)  # per-row exp(ba)
        sel64 = np.ascontiguousarray(
            np.broadcast_to(
                np.diag(sexp).astype(np.float32), (128, 64, 64)
            ).reshape(128, 4096)
        ).astype(bf)
        for j, ch in enumerate(chs):
            w = 8 * ch
            nw = WIN_OF_J[j]
            arr = Wd5[:, ch, :, :ch, :].reshape(CDIM, FOUT, w)  # [k, q, w]
            arrp = np.zeros((CDIM, FOUT, 128 * nw), dtype=np.float32)
            arrp[:, :, :w] = arr
            bl = arrp.reshape(CDIM, FOUT, nw, 128).transpose(1, 2, 3, 0)
            blocks.append(np.ascontiguousarray(bl.reshape(FOUT * nw, 128, CDIM)))
            bdj = bd4[ch, :, :ch, :].reshape(FOUT, w)  # [q, w]
            if w >= 128:
                a64 = arr.astype(np.float64)
                G = np.matmul(a64.transpose(1, 0, 2), a64.transpose(1, 2, 0))
                tr = np.trace(G, axis1=1, axis2=2)
                G += np.eye(CDIM)[None] * (1e-9 * tr[:, None, None] / CDIM)
                L = np.linalg.cholesky(G)  # [q, 128, 128], G = L @ L.T
                for q in range(FOUT):
                    lch[:, j * 8 + q, :] = L[q]
            else:
                for q in range(FOUT):
                    lch[:, j * 8 + q, :w] = arr[:, q, :]
            for q in range(FOUT):
                r = j * 8 + q
                crossq[:, r] = 2.0 * (arr[:, q, :] @ bdj[q])
                ebc[r] = bdj[q] @ bdj[q]
                bdm[:w, r] = bdj[q] * np.exp(ba[ch * FOUT + q])
                wdd[:, r * 8 : (r + 1) * 8] = Wd5[:, ch, q, ch, :]
                bdd[r * 8 : (r + 1) * 8] = bd4[ch, q, ch, :]
                sb_r = np.exp(ba[ch * FOUT + q]).astype(np.float32)
                xvd_cols[r * 8 : (r + 1) * 8, :] = (
                    xvT[8 * ch : 8 * ch + 8, :] * sb_r
                )
                exl_cols[r * 8 : (r + 1) * 8, :] = (
                    exlT[8 * ch : 8 * ch + 8, :] * sb_r
                )
            rows = slice(ch * FOUT, (ch + 1) * FOUT)
            wampq[:, j * 8 : (j + 1) * 8] = Wa[:, rows]
            biasq[:, j * 8 : (j + 1) * 8] = Wb[:, rows]
            eba[j * 8 : (j + 1) * 8] = ba[rows]
            ebb[j * 8 : (j + 1) * 8] = bb[rows]

        wtk = np.concatenate(blocks, axis=0)  # [160, 128, 128] (blk, i, k)
        wtk = np.ascontiguousarray(
            wtk.transpose(1, 0, 2).reshape(128, NBLK * 128)
        )

        warm = np.zeros((128, WM_END), dtype=np.float32)
        warm[:, WM_CT : WM_CT + 256] = cT
        warm[:, WM_WDD : WM_WDD + 512] = wdd
        warm[:, WM_WAMP : WM_WAMP + 64] = wampq
        warm[:, WM_BIAS : WM_BIAS + 64] = biasq
        for a in range(4):
            warm[:, WM_BDT + a] = bdd[128 * a : 128 * (a + 1)]
            warm[:, WM_BDT + 4 + a] = 2.0 * bdd[128 * a : 128 * (a + 1)]
        warm[0, WM_SCL : WM_SCL + 64] = ebc
        warm[0, WM_SCL + 64 : WM_SCL + 128] = ebb

        dx2 = np.zeros((128, DX_END), dtype=np.float32)
        dx2[:, DX_XVD : DX_XVD + 1024] = fold4(xvd_cols)
        dx2[:, DX_EXL : DX_EXL + 1024] = fold4(exl_cols)
        dx2[:, DX_BDM : DX_BDM + 256] = fold4(bdm)
        dx2[:, DX_SEL16 : DX_SEL16 + 256] = sel16.reshape(128, 256)
        dx2[:, DX_CROSS : DX_CROSS + 64] = crossq
        dx2[0, DX_EBC : DX_EBC + 64] = ebc

        in_maps.append(
            {
                "warm": warm.astype(bf),
                "xvt": xvt,
                "wtk": wtk.astype(bf),
                "lch": np.ascontiguousarray(lch.reshape(128, NLOC * 128)).astype(bf),
                "sel64": sel64,
                "sel128": sel128,
                "dx2": dx2.astype(bf),
            }
        )
    return in_maps


def kernel(x, c, Wd, bd, Wa, ba, Wb, bb, _trace=False, _tmpdir=None):
    global _NC
    from concourse.bass_utils import run_bass_kernel_spmd

    if _NC is None:
        _NC = _build_nc()
    in_maps = _host_prep(x, c, Wd, bd, Wa, ba, Wb, bb)
    res = run_bass_kernel_spmd(
        _NC, in_maps, core_ids=list(range(NCORES)), trace=_trace, tmpdir=_tmpdir
    )

    out = np.empty((B, O, 2), dtype=np.float32)
    for k in range(NCORES):
        ok = res.results[k]["out"]
        for j, ch in enumerate(_channels(k)):
            out[:, ch * FOUT : (ch + 1) * FOUT, :] = ok[:, j * FOUT : (j + 1) * FOUT, :]
    if _trace:
        return out, res
    return out
